# revision 1
# baseline (speedup 1.0000x reference)
"""Trainium2 Bass kernel for a 3-layer GCN encoder (B=32, N=1000, D=256).

Math: the reference's normalized adjacency for a fully-connected graph
(self_loop=False -> adj = ones) is A_norm = ones(N,N)/N, so the
"aggregation" einsum is a mean over nodes broadcast back to every node.
Since mean o linear = linear o mean and the mean is idempotent across
layers (h is constant over nodes after layer 0), the whole network
collapses to, per batch b:

    m_b  = mean_n node_feature[b, n, :]          # (D,)
    h1_b = relu(m_b @ W0 + b0)
    h2_b = relu(h1_b @ W1 + b1)
    h3_b = h2_b @ W2 + b2
    out[b, n, :] = node_feature[b, n, :] + h3_b  # broadcast residual

Sharding: data-parallel over batch, 4 batches per core on 8 cores.

Per-core dataflow (all HW-tuned via A/B benching on the axon trn2 pool):
- Loads are split in halves across BOTH HWDGE rings (SP + ACT) so the two
  rings run concurrently; stores go out via the SWDGE (gpsimd) path (plus
  HWDGE halves), keeping every DMA queue single-direction — mixing
  directions on one queue causes head-of-line blocking at the sequencer.
- Per-batch column sums run on the PE (data as stationary, ones vector
  moving, PSUM accumulation), the 256x256 chain runs in transposed
  orientation (weights as stationary, h as a 1-column moving operand),
  bias+relu is a single DVE tensor_scalar op, the h3 broadcast across
  partitions is a rank-1 PE matmul, and the residual add is an in-place
  DVE tensor_tensor. No compute ever lands on the DMA-issuing engines
  (SP/ACT/gpsimd), which benchmarks showed starves the DMA queues.
"""

import numpy as np

import concourse.bacc as bacc
import concourse.bass as bass
import concourse.mybir as mybir
import concourse.tile as tile
from concourse.bass_utils import run_bass_kernel_spmd

F32 = mybir.dt.float32

B, N, D, L = 32, 1000, 256, 3
NCORES = 8
NB = B // NCORES  # batches per core
P = 125           # partition rows per node-slice
T = N // P        # node-slices per batch
HALF = 128        # half of D (partition dim for transposed chain)

# DMA queue assignment (per batch): loads split in halves across the two
# HWDGE rings; stores mostly on the SWDGE (gpsimd) queue.
LOAD_ENGINES = [["sync", "scalar"]] * NB

_NC_CACHE = {}


def _build_nc(reps=1):
    nc = bacc.Bacc("TRN2", target_bir_lowering=False, debug=False)

    nf_d = nc.dram_tensor("nf", [NB, N, D], F32, kind="ExternalInput")
    w_d = nc.dram_tensor("w", [L, D, D], F32, kind="ExternalInput")
    bvec_d = nc.dram_tensor("bvec", [HALF, 2 * L], F32, kind="ExternalInput")
    out_d = nc.dram_tensor("out", [NB, N, D], F32, kind="ExternalOutput")

    ones_col_d = nc.inline_tensor(np.ones((P, 1), np.float32), "ones_col")
    ones_row_d = nc.inline_tensor(np.ones((1, P), np.float32), "ones_row")
    ident_d = nc.inline_tensor(np.eye(HALF, dtype=np.float32), "ident")

    add_op = mybir.AluOpType.add
    max_op = mybir.AluOpType.max

    with tile.TileContext(nc) as tc:
        with (
            tc.tile_pool(name="const", bufs=1) as cpool,
            tc.tile_pool(name="data", bufs=NB) as dpool,
            tc.tile_pool(name="vec", bufs=8) as vpool,
            tc.tile_pool(name="ps_sum", bufs=2, space=bass.MemorySpace.PSUM) as ps_sum,
            tc.tile_pool(name="ps_chain", bufs=2, space=bass.MemorySpace.PSUM) as ps_chain,
            tc.tile_pool(name="ps_row", bufs=1, space=bass.MemorySpace.PSUM) as ps_row,
            tc.tile_pool(name="ps_bc", bufs=3, space=bass.MemorySpace.PSUM) as ps_bc,
        ):
            # ---- constants ----
            w_sb = []
            for l in range(L):
                wt = cpool.tile([HALF, 2, D], F32, tag=f"w{l}", name=f"w{l}")
                nc.sync.dma_start(wt[:], w_d[l].rearrange("(kc k) e -> k kc e", k=HALF))
                w_sb.append(wt)
            bvec = cpool.tile([HALF, 2 * L], F32, tag="bvec", name="bvec")
            nc.sync.dma_start(bvec[:], bvec_d[:])
            ones_col = cpool.tile([P, 1], F32, tag="ones_col", name="ones_col")
            nc.sync.dma_start(ones_col[:], ones_col_d[:])
            ones_row = cpool.tile([1, P], F32, tag="ones_row", name="ones_row")
            nc.sync.dma_start(ones_row[:], ones_row_d[:])
            ident = cpool.tile([HALF, HALF], F32, tag="ident", name="ident")
            nc.sync.dma_start(ident[:], ident_d[:])

            def batch_body():
                for b in range(NB):
                    nf_t = dpool.tile([P, T, D], F32, tag="nf", name=f"nf{b}")
                    src = nf_d[b].rearrange("(t p) d -> p t d", p=P)
                    spec = LOAD_ENGINES[b]
                    step = T // len(spec)
                    for s, eng in enumerate(spec):
                        getattr(nc, eng).dma_start(
                            nf_t[:, s * step:(s + 1) * step, :],
                            src[:, s * step:(s + 1) * step, :],
                        )

                    # per-batch column sums (transposed orientation):
                    # sumT[mh][d, 0] = sum_n nf[b, n, mh*128 + d]
                    h = []
                    for mh in range(2):
                        ps = ps_sum.tile([HALF, 1], F32, tag="ps_s", name=f"ps_s{b}_{mh}")
                        for t in range(T):
                            nc.tensor.matmul(
                                ps[:],
                                nf_t[:, t, mh * HALF:(mh + 1) * HALF],
                                ones_col[:],
                                start=(t == 0),
                                stop=(t == T - 1),
                            )
                        s = vpool.tile([HALF, 1], F32, tag="hT", name=f"sum{b}_{mh}")
                        nc.vector.tensor_scalar_mul(s[:], ps[:], 1.0 / N)
                        h.append(s)

                    # 3-layer chain, transposed orientation, bias+relu on DVE
                    for l in range(L):
                        hn = []
                        for mh in range(2):
                            pc = ps_chain.tile(
                                [HALF, 1], F32, tag="ps_c", name=f"ps_c{b}_{l}_{mh}"
                            )
                            for kc in range(2):
                                nc.tensor.matmul(
                                    pc[:],
                                    w_sb[l][:, kc, mh * HALF:(mh + 1) * HALF],
                                    h[kc][:],
                                    start=(kc == 0),
                                    stop=(kc == 1),
                                )
                            ht = vpool.tile([HALF, 1], F32, tag="hT", name=f"h{b}_{l}_{mh}")
                            bias_ap = bvec[:, 2 * l + mh:2 * l + mh + 1]
                            if l < L - 1:
                                nc.vector.tensor_scalar(
                                    ht[:], pc[:], bias_ap, 0.0, add_op, max_op
                                )
                            else:
                                nc.vector.tensor_scalar_add(ht[:], pc[:], bias_ap)
                            hn.append(ht)
                        h = hn

                    # transpose h3 back to a row, broadcast across partitions
                    pr = ps_row.tile([1, D], F32, tag="ps_r", name=f"ps_r{b}")
                    for kc in range(2):
                        nc.tensor.transpose(
                            pr[0:1, kc * HALF:(kc + 1) * HALF], h[kc][:], ident[:]
                        )
                    h3row = vpool.tile([1, D], F32, tag="h3row", name=f"h3row{b}")
                    nc.vector.tensor_copy(h3row[:], pr[:])
                    pb = ps_bc.tile([P, D], F32, tag="ps_b", name=f"ps_b{b}")
                    nc.tensor.matmul(pb[:], ones_row[:], h3row[:], start=True, stop=True)

                    # residual add in place, reading the broadcast straight from
                    # PSUM (fp32 tensor_tensor is 1x regardless, so the PSUM
                    # operand is free and saves a copy); store each half as soon
                    # as its adds complete
                    dst = out_d[b].rearrange("(t p) d -> p t d", p=P)
                    for half in range(2):
                        for t in range(half * 4, half * 4 + 4):
                            nc.vector.tensor_add(nf_t[:, t, :], nf_t[:, t, :], pb[:])
                        nc.gpsimd.dma_start(
                            dst[:, half * 4:half * 4 + 4, :],
                            nf_t[:, half * 4:half * 4 + 4, :],
                        )

            if reps == 1:
                batch_body()
            else:
                with tc.For_i(0, reps, 1):
                    batch_body()

    nc.compile()
    return nc


def _get_nc(reps=1):
    if reps not in _NC_CACHE:
        _NC_CACHE[reps] = _build_nc(reps)
    return _NC_CACHE[reps]


def _make_in_maps(node_feature, Ws, bs):
    nf = np.ascontiguousarray(np.asarray(node_feature, dtype=np.float32))
    w = np.ascontiguousarray(np.asarray(Ws, dtype=np.float32))
    b = np.asarray(bs, dtype=np.float32)
    # bvec[p, 2*l + half] = bs[l, half*128 + p]
    bvec = np.ascontiguousarray(
        b.reshape(L, 2, HALF).transpose(2, 0, 1).reshape(HALF, 2 * L)
    )
    in_maps = []
    for i in range(NCORES):
        in_maps.append(
            {
                "nf": np.ascontiguousarray(nf[i * NB:(i + 1) * NB]),
                "w": w,
                "bvec": bvec,
            }
        )
    return in_maps


def run_on_hw(node_feature, Ws, bs):
    import os

    # The NTFF trace hook (antenv.axon_hooks) does not exist in this
    # container; make sure an inherited BASS_TRACE can't pull it in.
    os.environ["BASS_NEVER_TRACE"] = "1"
    nc = _get_nc()
    res = run_bass_kernel_spmd(
        nc,
        _make_in_maps(node_feature, Ws, bs),
        list(range(NCORES)),
        trace=False,
    )
    out = np.concatenate([res.results[i]["out"] for i in range(NCORES)], axis=0)
    return out, res


def kernel(x, node_feature, Ws, bs):
    node_feature = np.asarray(node_feature, dtype=np.float32)
    out, _ = run_on_hw(node_feature, Ws, bs)
    return out, node_feature


# ---------------------------------------------------------------------------
# Timing runner: same PJRT path as run_bass_kernel_spmd under axon, but with
# the jitted executable cached so repeated executions can be timed without
# re-tracing/re-compiling. Used by test.py only.
# ---------------------------------------------------------------------------


class _Runner:
    def __init__(self, nc=None):
        import jax
        from jax.experimental.shard_map import shard_map
        from jax.sharding import Mesh, NamedSharding, PartitionSpec

        from concourse.bass2jax import (
            _bass_exec_p,
            install_neuronx_cc_hook,
            partition_id_tensor,
        )

        install_neuronx_cc_hook()
        self.jax = jax
        if nc is None:
            nc = _get_nc(1)
        partition_name = (
            nc.partition_id_tensor.name if nc.partition_id_tensor else None
        )
        in_names, out_names, out_avals, zero_outs = [], [], [], []
        for alloc in nc.m.functions[0].allocations:
            if not isinstance(alloc, mybir.MemoryLocationSet):
                continue
            name = alloc.memorylocations[0].name
            if alloc.kind == "ExternalInput":
                if name != partition_name:
                    in_names.append(name)
            elif alloc.kind == "ExternalOutput":
                shape = tuple(alloc.tensor_shape)
                dt = mybir.dt.np(alloc.dtype)
                out_names.append(name)
                out_avals.append(jax.core.ShapedArray(shape, dt))
                zero_outs.append(np.zeros(shape, dt))
        self.in_names = in_names
        self.out_names = out_names
        self.out_avals = out_avals
        self.zero_outs = zero_outs
        n_params, n_outs = len(in_names), len(out_names)
        all_names = tuple(
            in_names + out_names + ([partition_name] if partition_name else [])
        )

        def _body(*args):
            operands = list(args)
            if partition_name is not None:
                operands.append(partition_id_tensor())
            outs = _bass_exec_p.bind(
                *operands,
                out_avals=tuple(out_avals),
                in_names=all_names,
                out_names=tuple(out_names),
                lowering_input_output_aliases=(),
                sim_require_finite=True,
                sim_require_nnan=True,
                nc=nc,
            )
            return tuple(outs)

        devices = jax.devices()[:NCORES]
        self.mesh = Mesh(np.asarray(devices), ("core",))
        self.sharding = NamedSharding(self.mesh, PartitionSpec("core"))
        in_specs = (PartitionSpec("core"),) * (n_params + n_outs)
        out_specs = (PartitionSpec("core"),) * n_outs
        self.jitted = jax.jit(
            shard_map(
                _body,
                mesh=self.mesh,
                in_specs=in_specs,
                out_specs=out_specs,
                check_rep=False,
            ),
            donate_argnums=tuple(range(n_params, n_params + n_outs)),
            keep_unused=True,
        )

    def stage_inputs(self, in_maps):
        concat = [
            np.concatenate([m[name] for m in in_maps], axis=0)
            for name in self.in_names
        ]
        return [self.jax.device_put(a, self.sharding) for a in concat]

    def stage_zeros(self):
        return [
            self.jax.device_put(
                np.zeros((NCORES * z.shape[0], *z.shape[1:]), z.dtype), self.sharding
            )
            for z in self.zero_outs
        ]

    def run(self, dev_inputs, dev_zeros):
        return self.jitted(*dev_inputs, *dev_zeros)


_RUNNER_CACHE = {}


def get_runner(reps=1):
    if reps not in _RUNNER_CACHE:
        _RUNNER_CACHE[reps] = _Runner(_get_nc(reps))
    return _RUNNER_CACHE[reps]



# revision 28
# speedup vs baseline: 1.6914x; 1.6914x over previous
"""Trainium2 Bass kernel for a 3-layer GCN encoder (B=32, N=1000, D=256).

Math: the reference's normalized adjacency for a fully-connected graph
(self_loop=False -> adj = ones) is A_norm = ones(N,N)/N, so the
"aggregation" einsum is a mean over nodes broadcast back to every node.
Since mean o linear = linear o mean and the mean is idempotent across
layers (h is constant over nodes after layer 0), the whole network
collapses to, per batch b:

    m_b  = mean_n node_feature[b, n, :]          # (D,)
    h1_b = relu(m_b @ W0 + b0)
    h2_b = relu(h1_b @ W1 + b1)
    h3_b = h2_b @ W2 + b2
    out[b, n, :] = node_feature[b, n, :] + h3_b  # broadcast residual

Sharding: data-parallel over batch, 4 batches per core on 8 cores.

v2 dataflow (per core):
- Node split "(p t)": partition p holds nodes 8p..8p+7, so every DMA
  descriptor is one contiguous 8KB (f32) / 4KB (f16) run and consecutive
  descriptors tile the batch region sequentially in HBM.
- Loads: one 1MB SWDGE (gpsimd) DMA per batch that casts f32->f16 in
  flight (halves SBUF traffic, enables DVE 4x perf mode downstream).
- Chain compute in f16 (weights pre-cast and W0 pre-scaled by 1/N on
  host), PSUM accumulation in f32, bias+relu on DVE.
- Residual add: rank-1 PE broadcast of h3 -> PSUM, DVE copy to f16
  SBUF, then in-place all-f16 SBUF tensor_adds (DVE 2x/4x mode).
- Stores: one 512KB f16 HWDGE DMA per batch (sync/scalar rings carry
  only stores; loads ride SWDGE, so every queue is single-direction).
- Output is f16 on device; host upcasts to float32 (rel err ~1e-3,
  tolerance is 2e-2).
"""

import numpy as np

import concourse.bacc as bacc
import concourse.bass as bass
import concourse.mybir as mybir
import concourse.tile as tile
from concourse.bass_utils import run_bass_kernel_spmd

F32 = mybir.dt.float32
F16 = mybir.dt.float16

B, N, D, L = 32, 1000, 256, 3
NCORES = 8
NB = B // NCORES  # batches per core
P = 125           # partition rows per node-slice
T = N // P        # node-slices per partition row
HALF = 128        # half of D (partition dim for transposed chain)

# tuning knobs (sim/HW A/B)
POOL_ADD_SLICES = 2  # residual-add t-slices per batch offloaded to gpsimd

_NC_CACHE = {}


def _build_nc(reps=1):
    nc = bacc.Bacc("TRN2", target_bir_lowering=False, debug=False)

    nf_d = nc.dram_tensor("nf", [NB, N, D], F32, kind="ExternalInput")
    w_d = nc.dram_tensor("w", [L, D, D], F16, kind="ExternalInput")
    bvec_d = nc.dram_tensor("bvec", [HALF, 2 * L], F32, kind="ExternalInput")
    out_d = nc.dram_tensor("out", [NB, N, D], F16, kind="ExternalOutput")

    # sel[:, mh, :]: stationary for the per-half broadcast (row mh ones)
    sel_np = np.zeros((2, 2, P), np.float16)
    for mh in range(2):
        sel_np[mh, mh, :] = 1.0
    sel_d = nc.inline_tensor(sel_np, "sel")
    ident_d = nc.inline_tensor(np.eye(HALF, dtype=np.float32), "ident")

    COPY = mybir.ActivationFunctionType.Copy
    IDENT = mybir.ActivationFunctionType.Identity
    RELU = mybir.ActivationFunctionType.Relu

    with tile.TileContext(nc) as tc:
        with (
            tc.tile_pool(name="const", bufs=1) as cpool,
            tc.tile_pool(name="data", bufs=NB) as dpool,
            tc.tile_pool(name="vec", bufs=4) as vpool,
            tc.tile_pool(name="ps_sum", bufs=2, space=bass.MemorySpace.PSUM) as ps_sum,
            tc.tile_pool(name="ps_chain", bufs=2, space=bass.MemorySpace.PSUM) as ps_chain,
            tc.tile_pool(name="ps_row", bufs=2, space=bass.MemorySpace.PSUM) as ps_row,
            tc.tile_pool(name="ps_bc", bufs=2, space=bass.MemorySpace.PSUM) as ps_bc,
        ):
            # ---- constants ----
            # ones/sel are memset on DVE (no DMA, ready ~instantly); the DMA
            # consts go smallest-first so they clear the DMA queue before the
            # 1MB loads monopolize it.
            ones_col = cpool.tile([P, 1], F16, tag="ones_col", name="ones_col")
            nc.vector.memset(ones_col[:], 1.0)
            sel = cpool.tile([2, 2, P], F16, tag="sel", name="sel")
            nc.sync.dma_start(sel[:], sel_d[:])
            bvec = cpool.tile([HALF, 2 * L], F32, tag="bvec", name="bvec")
            nc.sync.dma_start(bvec[:], bvec_d[:])
            ident = cpool.tile([HALF, HALF], F32, tag="ident", name="ident")
            nc.sync.dma_start(ident[:], ident_d[:])
            w_sb = []
            for l in range(L):
                wt = cpool.tile([HALF, 2, D], F16, tag=f"w{l}", name=f"w{l}")
                eng = nc.sync if l == 0 else nc.scalar
                eng.dma_start(wt[:], w_d[l].rearrange("(kc k) e -> k kc e", k=HALF))
                w_sb.append(wt)

            def batch_body():
                for b in range(NB):
                    # f32 -> f16 cast-on-load via SWDGE; contiguous 8KB reads
                    nf_t = dpool.tile([P, T, D], F16, tag="nf", name=f"nf{b}")
                    src = nf_d[b].rearrange("(p t) d -> p t d", p=P)
                    nc.gpsimd.dma_start(nf_t[:], src)

                    # column sums: ps_s[k, mh] = sum_n nf[b, n, mh*128+k]
                    ps_s = ps_sum.tile([HALF, 2], F32, tag="ps_s", name=f"ps_s{b}")
                    for mh in range(2):
                        for t in range(T):
                            nc.tensor.matmul(
                                ps_s[:, mh:mh + 1],
                                nf_t[:, t, mh * HALF:(mh + 1) * HALF],
                                ones_col[:],
                                start=(t == 0),
                                stop=(t == T - 1),
                            )
                    hc = vpool.tile([HALF, 2], F16, tag="h", name=f"sum{b}")
                    nc.scalar.activation(hc[:], ps_s[:], COPY)

                    # 3-layer chain: PE matmuls, ACT bias(+relu) per column
                    for l in range(L):
                        last = l == L - 1
                        pc = ps_chain.tile(
                            [HALF, 2], F32, tag="ps_c", name=f"ps_c{b}_{l}"
                        )
                        for mh in range(2):
                            for kc in range(2):
                                nc.tensor.matmul(
                                    pc[:, mh:mh + 1],
                                    w_sb[l][:, kc, mh * HALF:(mh + 1) * HALF],
                                    hc[:, kc:kc + 1],
                                    start=(kc == 0),
                                    stop=(kc == 1),
                                )
                        hn = vpool.tile(
                            [HALF, 2], F32 if last else F16,
                            tag="hT3" if last else "h", name=f"h{b}_{l}",
                        )
                        for mh in range(2):
                            nc.scalar.activation(
                                hn[:, mh:mh + 1],
                                pc[:, mh:mh + 1],
                                IDENT if last else RELU,
                                bias=bvec[:, 2 * l + mh:2 * l + mh + 1],
                            )
                        hc = hn

                    # h3 columns [128, 2] -> rows [2, 128] (row mh = half mh)
                    pr = ps_row.tile([2, HALF], F32, tag="ps_r", name=f"ps_r{b}")
                    nc.tensor.transpose(pr[:], hc[:], ident[:])
                    h3r = vpool.tile([2, HALF], F16, tag="h3r", name=f"h3r{b}")
                    nc.scalar.activation(h3r[:], pr[:], COPY)

                    # rank-1 broadcast of each half across all partitions
                    pb = ps_bc.tile([P, D], F32, tag="ps_b", name=f"ps_b{b}")
                    for mh in range(2):
                        nc.tensor.matmul(
                            pb[:, mh * HALF:(mh + 1) * HALF],
                            sel[:, mh, :],
                            h3r[:],
                            start=True,
                            stop=True,
                        )
                    pb16 = vpool.tile([P, D], F16, tag="pb16", name=f"pb16{b}")
                    nc.scalar.activation(pb16[:], pb[:], COPY)

                    # in-place residual adds (DVE fast mode; tail on gpsimd),
                    # then one 512KB f16 store on the SP HWDGE ring
                    for t in range(T):
                        eng = nc.gpsimd if t >= T - POOL_ADD_SLICES else nc.vector
                        eng.tensor_add(nf_t[:, t, :], nf_t[:, t, :], pb16[:])
                    dst = out_d[b].rearrange("(p t) d -> p t d", p=P)
                    nc.sync.dma_start(dst, nf_t[:])

            if reps == 1:
                batch_body()
            else:
                with tc.For_i(0, reps, 1):
                    batch_body()

    nc.compile()
    return nc


def _get_nc(reps=1):
    if reps not in _NC_CACHE:
        _NC_CACHE[reps] = _build_nc(reps)
    return _NC_CACHE[reps]


def _make_in_maps(node_feature, Ws, bs):
    nf = np.ascontiguousarray(np.asarray(node_feature, dtype=np.float32))
    w = np.asarray(Ws, dtype=np.float32).copy()
    w[0] *= 1.0 / N  # fold the mean's 1/N into the first layer's weights
    w16 = np.ascontiguousarray(w.astype(np.float16))
    b = np.asarray(bs, dtype=np.float32)
    # bvec[p, 2*l + half] = bs[l, half*128 + p]
    bvec = np.ascontiguousarray(
        b.reshape(L, 2, HALF).transpose(2, 0, 1).reshape(HALF, 2 * L)
    )
    in_maps = []
    for i in range(NCORES):
        in_maps.append(
            {
                "nf": np.ascontiguousarray(nf[i * NB:(i + 1) * NB]),
                "w": w16,
                "bvec": bvec,
            }
        )
    return in_maps


def run_on_hw(node_feature, Ws, bs):
    import os

    # The NTFF trace hook (antenv.axon_hooks) does not exist in this
    # container; make sure an inherited BASS_TRACE can't pull it in.
    os.environ["BASS_NEVER_TRACE"] = "1"
    nc = _get_nc()
    res = run_bass_kernel_spmd(
        nc,
        _make_in_maps(node_feature, Ws, bs),
        list(range(NCORES)),
        trace=False,
    )
    out = np.concatenate(
        [np.asarray(res.results[i]["out"]) for i in range(NCORES)], axis=0
    ).astype(np.float32)
    return out, res


def kernel(x, node_feature, Ws, bs):
    node_feature = np.asarray(node_feature, dtype=np.float32)
    out, _ = run_on_hw(node_feature, Ws, bs)
    return out, node_feature


# ---------------------------------------------------------------------------
# Timing runner: same PJRT path as run_bass_kernel_spmd under axon, but with
# the jitted executable cached so repeated executions can be timed without
# re-tracing/re-compiling. Used by test.py only.
# ---------------------------------------------------------------------------


class _Runner:
    def __init__(self, nc=None):
        import jax
        from jax.experimental.shard_map import shard_map
        from jax.sharding import Mesh, NamedSharding, PartitionSpec

        from concourse.bass2jax import (
            _bass_exec_p,
            install_neuronx_cc_hook,
            partition_id_tensor,
        )

        install_neuronx_cc_hook()
        self.jax = jax
        if nc is None:
            nc = _get_nc(1)
        partition_name = (
            nc.partition_id_tensor.name if nc.partition_id_tensor else None
        )
        in_names, out_names, out_avals, zero_outs = [], [], [], []
        for alloc in nc.m.functions[0].allocations:
            if not isinstance(alloc, mybir.MemoryLocationSet):
                continue
            name = alloc.memorylocations[0].name
            if alloc.kind == "ExternalInput":
                if name != partition_name:
                    in_names.append(name)
            elif alloc.kind == "ExternalOutput":
                shape = tuple(alloc.tensor_shape)
                dt = mybir.dt.np(alloc.dtype)
                out_names.append(name)
                out_avals.append(jax.core.ShapedArray(shape, dt))
                zero_outs.append(np.zeros(shape, dt))
        self.in_names = in_names
        self.out_names = out_names
        self.out_avals = out_avals
        self.zero_outs = zero_outs
        n_params, n_outs = len(in_names), len(out_names)
        all_names = tuple(
            in_names + out_names + ([partition_name] if partition_name else [])
        )

        def _body(*args):
            operands = list(args)
            if partition_name is not None:
                operands.append(partition_id_tensor())
            outs = _bass_exec_p.bind(
                *operands,
                out_avals=tuple(out_avals),
                in_names=all_names,
                out_names=tuple(out_names),
                lowering_input_output_aliases=(),
                sim_require_finite=True,
                sim_require_nnan=True,
                nc=nc,
            )
            return tuple(outs)

        devices = jax.devices()[:NCORES]
        self.mesh = Mesh(np.asarray(devices), ("core",))
        self.sharding = NamedSharding(self.mesh, PartitionSpec("core"))
        in_specs = (PartitionSpec("core"),) * (n_params + n_outs)
        out_specs = (PartitionSpec("core"),) * n_outs
        self.jitted = jax.jit(
            shard_map(
                _body,
                mesh=self.mesh,
                in_specs=in_specs,
                out_specs=out_specs,
                check_rep=False,
            ),
            donate_argnums=tuple(range(n_params, n_params + n_outs)),
            keep_unused=True,
        )

    def stage_inputs(self, in_maps):
        concat = [
            np.concatenate([m[name] for m in in_maps], axis=0)
            for name in self.in_names
        ]
        return [self.jax.device_put(a, self.sharding) for a in concat]

    def stage_zeros(self):
        return [
            self.jax.device_put(
                np.zeros((NCORES * z.shape[0], *z.shape[1:]), z.dtype), self.sharding
            )
            for z in self.zero_outs
        ]

    def run(self, dev_inputs, dev_zeros):
        return self.jitted(*dev_inputs, *dev_zeros)


_RUNNER_CACHE = {}


def get_runner(reps=1):
    if reps not in _RUNNER_CACHE:
        _RUNNER_CACHE[reps] = _Runner(_get_nc(reps))
    return _RUNNER_CACHE[reps]


# revision 36
# speedup vs baseline: 20.1240x; 11.8979x over previous
"""Trainium2 Bass kernel for a 3-layer GCN encoder (B=32, N=1000, D=256).

Math: the reference's normalized adjacency for a fully-connected graph
(self_loop=False -> adj = ones) is A_norm = ones(N,N)/N, so the
"aggregation" einsum is a mean over nodes broadcast back to every node.
Since mean o linear = linear o mean and the mean is idempotent across
layers (h is constant over nodes after layer 0), the whole network
collapses to, per batch b:

    m_b  = mean_n node_feature[b, n, :]          # (D,)
    h1_b = relu(m_b @ W0 + b0)
    h2_b = relu(h1_b @ W1 + b1)
    h3_b = h2_b @ W2 + b2
    out[b, n, :] = node_feature[b, n, :] + h3_b  # broadcast residual

Sharding: data-parallel over batch, 4 batches per core on 8 cores.

v2 dataflow (per core):
- Node split "(p t)": partition p holds nodes 8p..8p+7, so every DMA
  descriptor is one contiguous 8KB (f32) / 4KB (f16) run and consecutive
  descriptors tile the batch region sequentially in HBM.
- Loads: one 1MB SWDGE (gpsimd) DMA per batch that casts f32->f16 in
  flight (halves SBUF traffic, enables DVE 4x perf mode downstream).
- Chain compute in f16 (weights pre-cast and W0 pre-scaled by 1/N on
  host), PSUM accumulation in f32, bias+relu on DVE.
- Residual add: rank-1 PE broadcast of h3 -> PSUM, DVE copy to f16
  SBUF, then in-place all-f16 SBUF tensor_adds (DVE 2x/4x mode).
- Stores: one 512KB f16 HWDGE DMA per batch (sync/scalar rings carry
  only stores; loads ride SWDGE, so every queue is single-direction).
- Output is f16 on device; host upcasts to float32 (rel err ~1e-3,
  tolerance is 2e-2).
"""

import numpy as np

import concourse.bacc as bacc
import concourse.bass as bass
import concourse.mybir as mybir
import concourse.tile as tile
from concourse.bass_utils import run_bass_kernel_spmd

F32 = mybir.dt.float32
F16 = mybir.dt.float16

B, N, D, L = 32, 1000, 256, 3
NCORES = 8
NB = B // NCORES  # batches per core
P = 125           # partition rows per node-slice
T = N // P        # node-slices per partition row
HALF = 128        # half of D (partition dim for transposed chain)

# tuning knobs (sim/HW A/B)
POOL_ADD_SLICES = 2  # residual-add t-slices per batch offloaded to gpsimd

_NC_CACHE = {}


def _build_nc(reps=1):
    nc = bacc.Bacc("TRN2", target_bir_lowering=False, debug=False)

    nf_d = nc.dram_tensor("nf", [NB, N, D], F32, kind="ExternalInput")
    w_d = nc.dram_tensor("w", [L, D, D], F16, kind="ExternalInput")
    bvec_d = nc.dram_tensor("bvec", [HALF, 2 * L], F32, kind="ExternalInput")
    b2row_d = nc.dram_tensor("b2row", [1, D], F16, kind="ExternalInput")
    out_d = nc.dram_tensor("out", [NB, N, D], F16, kind="ExternalOutput")

    ones_row_d = nc.inline_tensor(np.ones((1, P), np.float16), "ones_row")

    COPY = mybir.ActivationFunctionType.Copy
    RELU = mybir.ActivationFunctionType.Relu

    with tile.TileContext(nc) as tc:
        with (
            tc.tile_pool(name="const", bufs=1) as cpool,
            tc.tile_pool(name="data", bufs=NB) as dpool,
            tc.tile_pool(name="vec", bufs=4) as vpool,
            tc.tile_pool(name="ps_sum", bufs=2, space=bass.MemorySpace.PSUM) as ps_sum,
            tc.tile_pool(name="ps_chain", bufs=2, space=bass.MemorySpace.PSUM) as ps_chain,
            tc.tile_pool(name="ps_row", bufs=2, space=bass.MemorySpace.PSUM) as ps_row,
            tc.tile_pool(name="ps_bc", bufs=2, space=bass.MemorySpace.PSUM) as ps_bc,
        ):
            # ---- constants ----
            # ones/sel are memset on DVE (no DMA, ready ~instantly); the DMA
            # consts go smallest-first so they clear the DMA queue before the
            # 1MB loads monopolize it.
            ones_col = cpool.tile([P, 1], F16, tag="ones_col", name="ones_col")
            nc.vector.memset(ones_col[:], 1.0)
            ones_row = cpool.tile([1, P], F16, tag="ones_row", name="ones_row")
            nc.sync.dma_start(ones_row[:], ones_row_d[:])
            b2row = cpool.tile([1, D], F16, tag="b2row", name="b2row")
            nc.sync.dma_start(b2row[:], b2row_d[:])
            bvec = cpool.tile([HALF, 2 * L], F32, tag="bvec", name="bvec")
            nc.sync.dma_start(bvec[:], bvec_d[:])
            w_sb = []
            for l in range(L):
                wt = cpool.tile([HALF, 2, D], F16, tag=f"w{l}", name=f"w{l}")
                eng = nc.sync if l == 0 else nc.scalar
                eng.dma_start(wt[:], w_d[l].rearrange("(kc k) e -> k kc e", k=HALF))
                w_sb.append(wt)

            def batch_body():
                for b in range(NB):
                    # f32 -> f16 cast-on-load via SWDGE; contiguous 8KB reads.
                    # Loads are emitted at natural priority; all compute of
                    # batch b is floored (scheduler-only timestamps) so the
                    # emitted per-engine order is strictly batch-sequential —
                    # otherwise the list scheduler interleaves later batches'
                    # sums before earlier batches' chains, and the counting
                    # semaphores then serialize the whole tail.
                    tc.tile_set_cur_wait(0)
                    nf_t = dpool.tile([P, T, D], F16, tag="nf", name=f"nf{b}")
                    src = nf_d[b].rearrange("(p t) d -> p t d", p=P)
                    nc.gpsimd.dma_start(nf_t[:], src)
                    tc.tile_set_cur_wait(0.05 * b)

                    # column sums: ps_s[k, mh] = sum_n nf[b, n, mh*128+k]
                    ps_s = ps_sum.tile([HALF, 2], F32, tag="ps_s", name=f"ps_s{b}")
                    for mh in range(2):
                        for t in range(T):
                            nc.tensor.matmul(
                                ps_s[:, mh:mh + 1],
                                nf_t[:, t, mh * HALF:(mh + 1) * HALF],
                                ones_col[:],
                                start=(t == 0),
                                stop=(t == T - 1),
                            )
                    hc = vpool.tile([HALF, 2], F16, tag="h", name=f"sum{b}")
                    nc.scalar.activation(hc[:], ps_s[:], COPY)

                    # layers 0,1 in column orientation: PE matmul, ACT
                    # bias+relu per column
                    for l in range(L - 1):
                        pc = ps_chain.tile(
                            [HALF, 2], F32, tag="ps_c", name=f"ps_c{b}_{l}"
                        )
                        for mh in range(2):
                            for kc in range(2):
                                nc.tensor.matmul(
                                    pc[:, mh:mh + 1],
                                    w_sb[l][:, kc, mh * HALF:(mh + 1) * HALF],
                                    hc[:, kc:kc + 1],
                                    start=(kc == 0),
                                    stop=(kc == 1),
                                )
                        hn = vpool.tile([HALF, 2], F16, tag="h", name=f"h{b}_{l}")
                        for mh in range(2):
                            nc.scalar.activation(
                                hn[:, mh:mh + 1],
                                pc[:, mh:mh + 1],
                                RELU,
                                bias=bvec[:, 2 * l + mh:2 * l + mh + 1],
                            )
                        hc = hn

                    # layer 2 in row orientation: h3row = h2 @ W2 as [1, 256]
                    pr = ps_row.tile([1, D], F32, tag="ps_r", name=f"ps_r{b}")
                    for kc in range(2):
                        nc.tensor.matmul(
                            pr[:],
                            hc[:, kc:kc + 1],
                            w_sb[L - 1][:, kc, :],
                            start=(kc == 0),
                            stop=(kc == 1),
                        )
                    h3r = vpool.tile([1, D], F16, tag="h3r", name=f"h3r{b}")
                    nc.scalar.activation(h3r[:], pr[:], COPY)

                    # rank-1 broadcast across partitions, + b2 the same way
                    pb = ps_bc.tile([P, D], F32, tag="ps_b", name=f"ps_b{b}")
                    nc.tensor.matmul(pb[:], ones_row[:], h3r[:], start=True, stop=False)
                    nc.tensor.matmul(pb[:], ones_row[:], b2row[:], start=False, stop=True)
                    pb16 = vpool.tile([P, D], F16, tag="pb16", name=f"pb16{b}")
                    nc.scalar.activation(pb16[:], pb[:], COPY)

                    # in-place residual adds (DVE fast mode + gpsimd), halves
                    # stored as soon as their adds land (SP HWDGE ring)
                    dst = out_d[b].rearrange("(p t) d -> p t d", p=P)
                    for half in range(2):
                        for t in range(half * 4, half * 4 + 4):
                            eng = nc.gpsimd if t % 4 == 0 else nc.vector
                            eng.tensor_add(nf_t[:, t, :], nf_t[:, t, :], pb16[:])
                        nc.sync.dma_start(
                            dst[:, half * 4:half * 4 + 4, :],
                            nf_t[:, half * 4:half * 4 + 4, :],
                        )
                tc.tile_set_cur_wait(0)

            if reps == 1:
                batch_body()
            else:
                with tc.For_i(0, reps, 1):
                    batch_body()

    nc.compile()
    return nc


def _get_nc(reps=1):
    if reps not in _NC_CACHE:
        _NC_CACHE[reps] = _build_nc(reps)
    return _NC_CACHE[reps]


def _make_in_maps(node_feature, Ws, bs):
    nf = np.ascontiguousarray(np.asarray(node_feature, dtype=np.float32))
    w = np.asarray(Ws, dtype=np.float32).copy()
    w[0] *= 1.0 / N  # fold the mean's 1/N into the first layer's weights
    w16 = np.ascontiguousarray(w.astype(np.float16))
    b = np.asarray(bs, dtype=np.float32)
    # bvec[p, 2*l + half] = bs[l, half*128 + p]
    bvec = np.ascontiguousarray(
        b.reshape(L, 2, HALF).transpose(2, 0, 1).reshape(HALF, 2 * L)
    )
    b2row = np.ascontiguousarray(b[L - 1].reshape(1, D).astype(np.float16))
    in_maps = []
    for i in range(NCORES):
        in_maps.append(
            {
                "nf": np.ascontiguousarray(nf[i * NB:(i + 1) * NB]),
                "w": w16,
                "bvec": bvec,
                "b2row": b2row,
            }
        )
    return in_maps


def run_on_hw(node_feature, Ws, bs):
    import os

    # The NTFF trace hook (antenv.axon_hooks) does not exist in this
    # container; make sure an inherited BASS_TRACE can't pull it in.
    os.environ["BASS_NEVER_TRACE"] = "1"
    nc = _get_nc()
    res = run_bass_kernel_spmd(
        nc,
        _make_in_maps(node_feature, Ws, bs),
        list(range(NCORES)),
        trace=False,
    )
    out = np.concatenate(
        [np.asarray(res.results[i]["out"]) for i in range(NCORES)], axis=0
    ).astype(np.float32)
    return out, res


def kernel(x, node_feature, Ws, bs):
    node_feature = np.asarray(node_feature, dtype=np.float32)
    out, _ = run_on_hw(node_feature, Ws, bs)
    return out, node_feature


# ---------------------------------------------------------------------------
# Timing runner: same PJRT path as run_bass_kernel_spmd under axon, but with
# the jitted executable cached so repeated executions can be timed without
# re-tracing/re-compiling. Used by test.py only.
# ---------------------------------------------------------------------------


class _Runner:
    def __init__(self, nc=None):
        import jax
        from jax.experimental.shard_map import shard_map
        from jax.sharding import Mesh, NamedSharding, PartitionSpec

        from concourse.bass2jax import (
            _bass_exec_p,
            install_neuronx_cc_hook,
            partition_id_tensor,
        )

        install_neuronx_cc_hook()
        self.jax = jax
        if nc is None:
            nc = _get_nc(1)
        partition_name = (
            nc.partition_id_tensor.name if nc.partition_id_tensor else None
        )
        in_names, out_names, out_avals, zero_outs = [], [], [], []
        for alloc in nc.m.functions[0].allocations:
            if not isinstance(alloc, mybir.MemoryLocationSet):
                continue
            name = alloc.memorylocations[0].name
            if alloc.kind == "ExternalInput":
                if name != partition_name:
                    in_names.append(name)
            elif alloc.kind == "ExternalOutput":
                shape = tuple(alloc.tensor_shape)
                dt = mybir.dt.np(alloc.dtype)
                out_names.append(name)
                out_avals.append(jax.core.ShapedArray(shape, dt))
                zero_outs.append(np.zeros(shape, dt))
        self.in_names = in_names
        self.out_names = out_names
        self.out_avals = out_avals
        self.zero_outs = zero_outs
        n_params, n_outs = len(in_names), len(out_names)
        all_names = tuple(
            in_names + out_names + ([partition_name] if partition_name else [])
        )

        def _body(*args):
            operands = list(args)
            if partition_name is not None:
                operands.append(partition_id_tensor())
            outs = _bass_exec_p.bind(
                *operands,
                out_avals=tuple(out_avals),
                in_names=all_names,
                out_names=tuple(out_names),
                lowering_input_output_aliases=(),
                sim_require_finite=True,
                sim_require_nnan=True,
                nc=nc,
            )
            return tuple(outs)

        devices = jax.devices()[:NCORES]
        self.mesh = Mesh(np.asarray(devices), ("core",))
        self.sharding = NamedSharding(self.mesh, PartitionSpec("core"))
        in_specs = (PartitionSpec("core"),) * (n_params + n_outs)
        out_specs = (PartitionSpec("core"),) * n_outs
        self.jitted = jax.jit(
            shard_map(
                _body,
                mesh=self.mesh,
                in_specs=in_specs,
                out_specs=out_specs,
                check_rep=False,
            ),
            donate_argnums=tuple(range(n_params, n_params + n_outs)),
            keep_unused=True,
        )

    def stage_inputs(self, in_maps):
        concat = [
            np.concatenate([m[name] for m in in_maps], axis=0)
            for name in self.in_names
        ]
        return [self.jax.device_put(a, self.sharding) for a in concat]

    def stage_zeros(self):
        return [
            self.jax.device_put(
                np.zeros((NCORES * z.shape[0], *z.shape[1:]), z.dtype), self.sharding
            )
            for z in self.zero_outs
        ]

    def run(self, dev_inputs, dev_zeros):
        return self.jitted(*dev_inputs, *dev_zeros)


_RUNNER_CACHE = {}


def get_runner(reps=1):
    if reps not in _RUNNER_CACHE:
        _RUNNER_CACHE[reps] = _Runner(_get_nc(reps))
    return _RUNNER_CACHE[reps]
